# revision 14
# baseline (speedup 1.0000x reference)
"""Bilinear field-interaction kernel for Trainium2 (Bass/Tile).

Reference computation:
    vid = einsum("bfd,de->bfe", x, W)          # x: [B, F, D], W: [D, D]
    ii, jj = triu_indices(F, k=1)              # P = F*(F-1)/2 pairs, i < j
    out[b, p, :] = x[b, ii[p], :] * vid[b, jj[p], :]   # [B, P, D]

Strategy (data-parallel over batch, 8 NeuronCores, 256 rows each):
  - the host pre-casts x and W to bf16 (halves input HBM reads and
    makes the whole on-device pipeline single-dtype).
  - per 128-row batch tile: load x_bf naturally ([b partitions, f*d
    free]) in three field-pieces, high fields first; per group of 4
    fields j: TensorE-transpose x_bf[:, j, :] -> [d, b] into one
    [128, 512] PSUM tile, one ACT copy to SBUF, then 4 bf16
    matmuls(lhsT=x_j^T, rhs=W) into one [128, 512] PSUM tile and one
    ACT copy -> vid[:, jlo:jlo+4, :] in [b, e] layout.  bf16 matmuls
    are single-pass (fp32 runs a 2-pass LOW/HIGH pipeline that made
    vid production the tile-0 bottleneck); batching 4 fields per ACT
    copy amortizes the ~190-cycle ACT overhead.
  - pair products on VectorE in bf16 (2x perf mode vs 1x for fp32):
    for fixed i the pairs (i, j=i+1..F-1) are contiguous in the pair
    dim, so one tensor_tensor per i-segment with a stride-0 broadcast
    of x_bf[:, i, :] over the j-run.
  - output staged and written in bf16 (halves the dominant HBM write
    traffic: 102 MB -> 51 MB per core); the host upcasts to f32.
    Scale-relative error of the bf16 pipeline is ~6e-3 (gate: 2e-2).
  - outputs staged in SBUF chunks of CHUNK pairs and DMA'd out
    alternating across both HWDGE rings.
"""

import numpy as np

BATCH, F, D = 2048, 40, 128
NCORES = 8
BSHARD = BATCH // NCORES        # 256 batch rows per core
P = 128                         # SBUF partitions = batch-tile height
NPAIRS = F * (F - 1) // 2       # 780
CHUNK = 65                      # pairs per staged output chunk (780 = 12*65)

_cache = {}


def build_bass(bshard=BSHARD, f=F, chunk=CHUNK):
    """Build the single-core Bass program (same program runs SPMD on all cores)."""
    import concourse.bass as bass
    import concourse.mybir as mybir
    from concourse.masks import make_identity
    from concourse.tile import TileContext

    fp32 = mybir.dt.float32
    bf16 = mybir.dt.bfloat16
    npairs = f * (f - 1) // 2
    ntiles = bshard // P
    assert bshard % P == 0

    # i-segments of the pair axis: (pair_start, i); j runs i+1 .. f-1
    segs = []
    ps = 0
    for i in range(f - 1):
        segs.append((ps, i))
        ps += f - 1 - i
    assert ps == npairs

    # descending groups of (up to) 4 fields for the batched vid pipeline:
    # [39..36], [35..32], ..., [7..4], [3..1]
    jgroups = []
    j = f - 1
    while j >= 1:
        jgroups.append(list(range(j, max(j - 4, 0), -1)))
        j -= 4

    # x load pieces (field ranges), loaded high-to-low so the PE/product
    # pipelines start after a fraction of the input is resident
    xpieces = [(28, f), (14, 28), (0, 14)]

    nc = bass.Bass()
    x = nc.dram_tensor("x", [bshard, f, D], bf16, kind="ExternalInput")
    w = nc.dram_tensor("w", [D, D], bf16, kind="ExternalInput")
    out = nc.dram_tensor("out", [bshard, npairs, D], bf16, kind="ExternalOutput")

    with TileContext(nc) as tc:
        with (
            tc.tile_pool(name="consts", bufs=1) as consts,
            tc.tile_pool(name="xbf", bufs=2) as xbf_pool,
            tc.tile_pool(name="vid", bufs=2) as vid_pool,
            tc.tile_pool(name="xt", bufs=3) as xt_pool,
            tc.tile_pool(name="obuf", bufs=4) as obuf_pool,
            tc.tile_pool(
                name="ptch", bufs=ntiles * (len(xpieces) + (npairs // chunk + 2))
            ) as ptch_pool,
            tc.tile_pool(name="xtp", bufs=2, space="PSUM") as xtp_pool,
            tc.tile_pool(name="vps", bufs=2, space="PSUM") as vps_pool,
            tc.tile_pool(name="wups", bufs=1, space="PSUM") as wu_pool,
        ):
            # tile-0 x loads first: they ride the gpsimd SWDGE queue (its
            # DMASW completion lanes are disjoint from the 8 DMAHW lanes,
            # which must cycle over output DMAs only — an input load landing
            # on a DMAHW lane gives the next output DMA on that lane a
            # second, un-elidable wait that walrus rejects), and issuing
            # them before make_identity keeps the Q7 descriptor work off
            # the first load's critical path.
            x_tiles = [
                xbf_pool.tile([P, f, D], bf16, tag=f"x_bf{t}", name=f"x_bf{t}")
                for t in range(ntiles)
            ]
            for pi, (flo, fhi) in enumerate(xpieces):
                # the first piece rides the sync HWDGE ring for its ~0.6us
                # first-byte latency (it gates the whole ramp); its DMAHW
                # lane is safe because the DVE pre-touch observes it
                ring = nc.sync if pi == 0 else nc.gpsimd
                ring.dma_start(x_tiles[0][:, flo:fhi, :], x[0:P, flo:fhi, :])

            ident = consts.tile([P, P], bf16)
            make_identity(nc, ident)
            w_sb = consts.tile([D, D], bf16)
            nc.scalar.dma_start(w_sb[:], w[:, :])
            # DVE-written scratch used as the source of post-touch copies
            # (reading it never pulls a non-DVE semaphore lane).
            pt_src = consts.tile([P, 1], bf16)
            nc.vector.memset(pt_src[:], 0.0)

            # PE warm-ups: touch the identity (Pool-produced) and W (DMA-
            # produced) once so later matmuls never need more than one new
            # semaphore wait — the PE LoadWeights command has a single wait
            # slot and walrus rejects matmuls with two pending waits.
            wu_ps = wu_pool.tile([P, D], bf16, tag="wu_t")
            nc.tensor.transpose(wu_ps[:], ident[:], ident[:])
            wu2_ps = wu_pool.tile([P, D], fp32, tag="wu_m")
            nc.tensor.matmul(wu2_ps[:], w_sb[:], ident[:], start=True, stop=True)
            wu_sb = consts.tile([P, 1], fp32)
            nc.scalar.copy(wu_sb[:], wu2_ps[:, 0:1])

            last_bufs = []   # final output-staging tiles, for post-touch
            OBUF_BUFS = 4    # staging depth; post-touches must cover this many
            out_dma_i = [0]  # alternate output DMAs across both HWDGE rings

            for t in range(ntiles):
                x_bf = x_tiles[t]
                if t > 0:
                    for flo, fhi in xpieces:
                        nc.gpsimd.dma_start(
                            x_bf[:, flo:fhi, :], x[t * P:(t + 1) * P, flo:fhi, :]
                        )
                # DVE pre-touch of the first piece; the later pieces are
                # touched just-in-time in the chunk loop below so the DVE
                # never blocks on a load it doesn't need yet (walrus allows
                # one wait per command, so each product op must find its
                # x piece already observed on the DVE lane).
                ptch_x = ptch_pool.tile([P, 1], bf16, tag="ptch")
                nc.vector.tensor_copy(ptch_x[:], x_bf[:, xpieces[0][0], 0:1])
                touched = 1  # pieces [0..touched) are DVE-observed

                # vid[:, j, :] = x_tile[:, j, :] @ W, for j = 1..f-1 (j=0
                # unused).  Computed in DESCENDING j groups of 4: the chunk
                # loop below runs in reverse pair order, and later chunks
                # only read the high-j vid slices, so the pair products can
                # start long before the whole vid tile is done.  Per group:
                # 4 transposes into one PSUM bank, one ACT copy, 4 bf16
                # matmuls into another PSUM bank, one ACT copy (rounds the
                # fp32 accumulation to bf16).
                vid_sb = vid_pool.tile([P, f, D], bf16)
                for g in jgroups:
                    ng = len(g)
                    jlo = g[-1]
                    xt_ps = xtp_pool.tile([P, ng * D], bf16)
                    for k, jj in enumerate(g):
                        nc.tensor.transpose(
                            xt_ps[:, k * D:(k + 1) * D], x_bf[:, jj, :], ident[:]
                        )
                    xt_sb = xt_pool.tile([P, ng * D], bf16, tag="xt")
                    nc.scalar.copy(xt_sb[:], xt_ps[:])
                    v_ps = vps_pool.tile([P, ng * D], fp32)
                    for k, jj in enumerate(g):
                        col = jj - jlo
                        nc.tensor.matmul(
                            v_ps[:, col * D:(col + 1) * D],
                            xt_sb[:, k * D:(k + 1) * D],
                            w_sb[:],
                            start=True,
                            stop=True,
                        )
                    nc.scalar.copy(vid_sb[:, jlo:jlo + ng, :], v_ps[:])

                # Chunk grid, processed in reverse pair order.  For tile 0
                # the top (= first-processed) cells form a ramp ladder
                # aligned to the 4-field vid groups — cell k needs exactly
                # one more vid group than cell k-1, so the output stream
                # starts ~5us in instead of waiting for two full groups.
                # Each cell keeps a single DMA, so staging-slot reuse never
                # sees more than one WAR lane.
                cells = [(c0, min(chunk, npairs - c0))
                         for c0 in range(0, npairs, chunk)]
                if t == 0:
                    cells = cells[:-1] + [(715, 29), (744, 21), (765, 9), (774, 6)]
                    assert sum(ch for _, ch in cells) == npairs
                for c0, ch in reversed(cells):
                    # Newest vid tick this chunk reads = its lowest j =
                    # i(c0)+1.  A tiny DVE pre-touch of that slice absorbs
                    # the ACT wait so the chunk's product ops carry at most
                    # the output-staging WAR wait (walrus: one wait/command).
                    i_first = max(i for (s, i) in segs if s <= c0)
                    pieces = []
                    for (s, i) in segs:
                        seg_len = f - 1 - i
                        lo = max(s, c0)
                        hi = min(s + seg_len, c0 + ch)
                        if lo >= hi:
                            continue
                        pieces.append((i, (i + 1) + (lo - s), hi - lo, lo - c0))

                    # just-in-time DVE touches for newly needed x pieces
                    while touched < len(xpieces) and i_first < xpieces[touched][1]:
                        ptch_x = ptch_pool.tile([P, 1], bf16, tag="ptch")
                        nc.vector.tensor_copy(
                            ptch_x[:], x_bf[:, xpieces[touched][0], 0:1]
                        )
                        touched += 1

                    ptch_c = ptch_pool.tile([P, 1], bf16, tag="ptch")
                    nc.vector.tensor_copy(ptch_c[:], vid_sb[:, i_first + 1, 0:1])
                    buf = obuf_pool.tile([P, chunk, D], bf16, tag="buf")
                    for (i, j0, ln, o) in pieces:
                        nc.vector.tensor_tensor(
                            buf[:, o:o + ln, :],
                            vid_sb[:, j0:j0 + ln, :],
                            x_bf[:, i:i + 1, :].to_broadcast([P, ln, D]),
                            mybir.AluOpType.mult,
                        )
                    # Taper the kernel tail: the final tile's last two
                    # chunks are split into smaller DMAs so the pure-DMA
                    # drain after the last vector op is shorter.  (Safe only
                    # here: these staging slots are never reused, so the
                    # extra DMA-completion lanes land on the post-touches.)
                    if t == ntiles - 1 and c0 == 0:
                        nsplit = 3
                    elif t == ntiles - 1 and c0 == chunk:
                        nsplit = 2
                    else:
                        nsplit = 1
                    bounds = [ch * k // nsplit for k in range(nsplit + 1)]
                    for a, b in zip(bounds[:-1], bounds[1:]):
                        ring = nc.sync if out_dma_i[0] % 2 == 0 else nc.scalar
                        out_dma_i[0] += 1
                        ring.dma_start(
                            out[t * P:(t + 1) * P, c0 + a:c0 + b, :],
                            buf[:, a:b, :],
                        )
                    last_bufs = (last_bufs + [(buf, bounds[:-1])])[-OBUF_BUFS:]

            # Post-touches: write one element into each of the final two
            # output-staging tiles so DVE observes their DMA completions
            # (WAR).  The kernel-tail drain then needs only its DVE wait —
            # walrus permits a single wait per command.  Source is a DVE-
            # written scratch tile, so no new semaphore lane is pulled in.
            for b_, starts in last_bufs:
                for a in starts:
                    nc.vector.tensor_copy(b_[:, a, 0:1], pt_src[:])

    _strip_redundant_self_waits(nc)
    _elide_transitive_waits(nc)
    return nc


def _strip_redundant_self_waits(nc):
    """Drop semaphore waits that are trivially satisfied by same-engine
    program order.

    Tile's wait emission is per-proc minimal but not transitively minimal:
    it sometimes emits a wait on an instruction's *own* engine semaphore for
    a tick the engine has already passed by program order (engines execute
    their stream serially, in order).  Walrus rejects PE Matmult / ACT
    Activation commands with more than one pending wait, so these redundant
    self-waits are fatal at codegen time.  A wait on sem S at position p of
    engine E's stream is removable iff S is incremented exclusively by E's
    instructions and the cumulative increments before p already reach the
    wait value.

    Only applied to PE, ACT and DVE: single-pipeline in-order engines whose
    command structs walrus limits to one wait (DVE additionally drains its
    pipe between ops).  GpSimd (Pool) runs 8 Q7 cores concurrently, so its
    self-waits are real synchronization.  Semaphores whose increments ride on
    DMACopy/collective instructions complete asynchronously and are never
    treated as program-ordered.
    """
    SERIAL_ENGINES = {"EngineType.PE", "EngineType.Activation", "EngineType.DVE"}
    ASYNC_OPS = ("DMA", "Collective")
    fn = nc.m.functions[0]
    blocks = list(fn.blocks)

    # sem -> set of engines that increment it
    inc_engines = {}
    for b in blocks:
        for inst in b.instructions:
            si = inst.sync_info
            if si is None:
                continue
            for u in si.on_update:
                if u.update_mode == "sem-inc":
                    src = str(inst.engine)
                    if any(m in str(inst.opcode) for m in ASYNC_OPS):
                        src = "ASYNC"
                    inc_engines.setdefault(u.ant_name, set()).add(src)

    cum = {}  # (engine, sem) -> incs seen so far in that engine's stream
    dropped = 0
    for b in blocks:
        for inst in b.instructions:
            eng = str(inst.engine)
            si = inst.sync_info
            if si is None:
                continue
            waits = list(si.on_wait)
            if waits:
                keep = []
                for w in waits:
                    if (
                        eng in SERIAL_ENGINES
                        and w.sync_type == "semaphore"
                        and w.wait_mode == "sem-ge-imm"
                        and inc_engines.get(w.ant_name) == {eng}
                        and cum.get((eng, w.ant_name), 0) >= w.wait_value
                    ):
                        dropped += 1
                        continue
                    keep.append(w)
                if len(keep) != len(waits):
                    si.on_wait = keep
                    inst.sync_info = si
            for u in si.on_update:
                if u.update_mode == "sem-inc":
                    k = (eng, u.ant_name)
                    cum[k] = cum.get(k, 0) + u.update_value
    return dropped


def _elide_transitive_waits(nc):
    """Drop semaphore waits already implied by an instruction's other waits
    (happens-before closure).

    Tile's wait emission is per-proc minimal at the instruction level but
    not transitively minimal, and this walrus build rejects any command
    with more than one pending wait.  Model:

      clock(X)   = knowledge guaranteed when X dispatches
                 = clock(engine-predecessor of X)            [dispatch order]
                 U for each wait (S >= v): {S: v} U release(producer(S, v))
      release(X) = clock(X) U X's own increments             [at inc-visibility]

    Engine-predecessor propagation uses only the predecessor's *dispatch*
    clock (its waits were satisfied before it issued), which is valid for
    every serial dispatch stream regardless of completion pipelining.  Pool
    (GpSimd, 8 concurrent cores) gets no predecessor propagation.  Any
    semaphore with a non-increment update is excluded entirely.

    A wait (S >= v) on a multi-wait instruction is dropped when the
    remaining waits plus predecessor knowledge already guarantee S >= v.
    """
    fn = nc.m.functions[0]
    insts = []
    for b in fn.blocks:
        insts.extend(b.instructions)

    # Positive sem-add-imm (HWDGE DMA completion) is an increment; anything
    # else (barrier dec/sub) disqualifies the semaphore from monotonic
    # reasoning.
    def inc_val(u):
        if u.update_mode == "sem-inc":
            return u.update_value
        if u.update_mode == "sem-add-imm" and u.update_value > 0:
            return u.update_value
        return None

    bad_sems = set()
    for inst in insts:
        si = inst.sync_info
        if si is None:
            continue
        for u in si.on_update:
            if inc_val(u) is None:
                bad_sems.add(u.ant_name)

    def join(dst, src):
        for k, v in src.items():
            if dst.get(k, 0) < v:
                dst[k] = v

    import bisect

    # Static producer map: sem -> sorted (cum_value_after_inc, inst_index).
    cum = {}
    producers = {}
    for idx, inst in enumerate(insts):
        si = inst.sync_info
        if si is None:
            continue
        for u in si.on_update:
            v = inc_val(u)
            if v is not None:
                cum[u.ant_name] = cum.get(u.ant_name, 0) + v
                producers.setdefault(u.ant_name, []).append((cum[u.ant_name], idx))

    release = [{} for _ in insts]  # knowledge when inst's incs are observed
    clocks = [{} for _ in insts]   # knowledge when inst dispatches

    def producer_release(sem, val):
        """Knowledge implied by having observed sem >= val (None if unknown)."""
        if sem in bad_sems:
            return None
        plist = producers.get(sem)
        if not plist or plist[-1][0] < val:
            return None
        k = bisect.bisect_left(plist, (val, -1))
        return release[plist[k][1]]

    def wait_knowledge(base, waits, skip=None):
        know = dict(base)
        for w in waits:
            if w is skip or w.sync_type != "semaphore" or w.wait_mode != "sem-ge-imm":
                continue
            know[w.ant_name] = max(know.get(w.ant_name, 0), w.wait_value)
            rel = producer_release(w.ant_name, w.wait_value)
            if rel:
                join(know, rel)
        return know

    # Fixpoint over happens-before (clocks only grow).
    for _ in range(6):
        cum2 = {}
        last_on_engine = {}
        for idx, inst in enumerate(insts):
            si = inst.sync_info
            eng = str(inst.engine)
            pred = last_on_engine.get(eng)
            pred_clock = {}
            if pred is not None and eng != "EngineType.Pool":
                pred_clock = clocks[pred]  # dispatch-order knowledge only
            waits = list(si.on_wait) if si is not None else []
            c = wait_knowledge(pred_clock, waits)
            r = dict(c)
            if si is not None:
                for u in si.on_update:
                    v = inc_val(u)
                    if v is not None:
                        cum2[u.ant_name] = cum2.get(u.ant_name, 0) + v
                        r[u.ant_name] = max(r.get(u.ant_name, 0), cum2[u.ant_name])
            clocks[idx] = c
            release[idx] = r
            last_on_engine[eng] = idx

    # Drop pass: remove waits implied by the instruction's other waits plus
    # engine-predecessor dispatch knowledge.
    dropped = 0
    last_on_engine = {}
    for idx, inst in enumerate(insts):
        si = inst.sync_info
        eng = str(inst.engine)
        pred = last_on_engine.get(eng)
        pred_clock = {}
        if pred is not None and eng != "EngineType.Pool":
            pred_clock = clocks[pred]
        waits = list(si.on_wait) if si is not None else []
        usable = [
            w for w in waits
            if w.sync_type == "semaphore" and w.wait_mode == "sem-ge-imm"
        ]
        if len(usable) >= 2 and len(usable) == len(waits):
            keep = list(usable)
            changed = True
            while changed and len(keep) > 1:
                changed = False
                for w in keep:
                    know = wait_knowledge(pred_clock, keep, skip=w)
                    if know.get(w.ant_name, 0) >= w.wait_value:
                        keep.remove(w)
                        dropped += 1
                        changed = True
                        break
            if len(keep) != len(waits):
                si.on_wait = keep
                inst.sync_info = si
        last_on_engine[eng] = idx
    return dropped


def _get_nc():
    if "nc" not in _cache:
        _cache["nc"] = build_bass()
    return _cache["nc"]


def make_in_maps(x: np.ndarray, W: np.ndarray):
    """Host-side prep: shard the batch and pre-cast inputs to bf16."""
    import ml_dtypes

    bf = ml_dtypes.bfloat16
    xb = np.asarray(x, dtype=np.float32).astype(bf)
    wb = np.ascontiguousarray(np.asarray(W, dtype=np.float32).astype(bf))
    return [
        {"x": np.ascontiguousarray(xb[c * BSHARD:(c + 1) * BSHARD]), "w": wb}
        for c in range(NCORES)
    ]


def kernel(x: np.ndarray, W: np.ndarray) -> np.ndarray:
    from concourse.bass_utils import run_bass_kernel_spmd

    nc = _get_nc()
    res = run_bass_kernel_spmd(nc, make_in_maps(x, W), list(range(NCORES)))
    return np.concatenate(
        [np.asarray(r["out"]).astype(np.float32) for r in res.results], axis=0
    )
